# revision 32
# baseline (speedup 1.0000x reference)
"""Multi-head attention (B=4, S=2048, d_model=1024, h=16) on 8 TRN2 NeuronCores.

Sharding: data-parallel over batch (4) x tensor-parallel over head-groups (2 x 8
heads, column-split Wq/Wk/Wv, row-split Wo). Each core computes a full (2048,
1024) bf16 partial of the output projection for its (batch, head-group); the
host upcasts, sums the two group partials per batch, and adds bo.

Device kernel (identical SPMD program on all 8 cores), all-bf16 matmuls:
  qT/kT = W @ X.T computed directly in head-major layout from RESIDENT x
  tiles (xq/xk live in SBUF for the whole kernel: one DMA each instead of a
  reload per pair — the repeated-x DMA was what starved pair 0's exp stream),
  scoresT = k @ qT per head with 64x128 row-tiled matmul pairs (the two heads
  run concurrently on the two halves of the PE array), one 1024-wide exp per
  double-buffered 2-bank PSUM scores block on the scalar engine (the ~285us
  ACT exp stream is the pacing floor; table-set preloaded during the
  DMA-bound head), AV as [v|1].T @ exps so the softmax denominators fall out
  of the matmul for free, normalization fused into the evacuation: a 1-lane
  approximate reciprocal read straight off the PSUM sums row, gpsimd
  partition-broadcast, one tensor_tensor multiply that reads the PSUM attn
  partial and writes normalized bf16 rows — then the output projection from
  the already-transposed attention output.

Scheduling (the engine queues are in-order, so emission order is the
schedule): v is projected in 256-wide half-chains — heads 0-3 just-in-time
inside pair 0's g-loop, heads 4-7 inside pair 1's — so pair 0's PE load
(~68us) fits under its 71us of exp instead of overflowing it by the full
v projection. q/k chains drip in 4-matmul halves emitted BEFORE the AV
matmuls each group, filling the PE while AV waits on exp. Output-projection
blocks drip after AV, split into a pairs-0..2 pre-chain and a pair-3 finish
two groups later, so no oproj matmul ever heads the PE queue waiting on the
just-emitted normalization — and the final block's pre-chains keep the PE
warm through the kernel tail.
"""
import ml_dtypes
import numpy as np

import concourse.bacc as bacc
import concourse.mybir as mybir
from concourse.tile import TileContext
from concourse.bass_utils import run_bass_kernel_spmd

P = 128
S = 2048          # sequence length
DM = 1024         # d_model
DG = 512          # dims per head-group (8 heads x 64)
NPAIR = 4         # head pairs per group
NQB = 4           # q blocks of 512
NKT = 16          # key tiles of 128
KT = DM // P      # contraction tiles for projections

F32 = mybir.dt.float32
BF16 = mybir.dt.bfloat16
AF = mybir.ActivationFunctionType


def _build(has_bias):
    nc = bacc.Bacc(None, target_bir_lowering=False)
    # all inputs are host-prepacked partition-major so every DMA reads
    # 4-8KB contiguous runs per partition (column-sliced layouts measured
    # at roughly half the aggregate DMA rate)
    xqT = nc.dram_tensor("xqT", [P, 4 * KT * DG], BF16, kind="ExternalInput")
    xkT = nc.dram_tensor("xkT", [P, 4 * KT * DG], BF16, kind="ExternalInput")
    xvT = nc.dram_tensor("xvT", [P, 4 * KT * DG], BF16, kind="ExternalInput")
    wqT = nc.dram_tensor("wqT", [P, KT * DG], BF16, kind="ExternalInput")
    wkT = nc.dram_tensor("wkT", [P, KT * DG], BF16, kind="ExternalInput")
    wvT = nc.dram_tensor("wvT", [P, KT * DG], BF16, kind="ExternalInput")
    woT = nc.dram_tensor("woT", [P, NPAIR * DM], BF16, kind="ExternalInput")
    if has_bias:
        bq = nc.dram_tensor("bq", [1, DG], BF16, kind="ExternalInput")
        bk = nc.dram_tensor("bk", [1, DG], BF16, kind="ExternalInput")
        bv = nc.dram_tensor("bv", [1, DG], BF16, kind="ExternalInput")
    out = nc.dram_tensor("out", [S, DM], BF16, kind="ExternalOutput")

    with TileContext(nc) as tc:
        with tc.tile_pool(name="pres", bufs=1) as pres, \
             tc.tile_pool(name="pw", bufs=3) as pw, \
             tc.tile_pool(name="pex", bufs=5) as pex, \
             tc.tile_pool(name="psmall", bufs=1) as psmall, \
             tc.tile_pool(name="pout", bufs=3) as pout, \
             tc.tile_pool(name="ps_proj", bufs=2, space="PSUM") as ps_proj, \
             tc.tile_pool(name="ps_sc", bufs=2, space="PSUM") as ps_sc, \
             tc.tile_pool(name="ps_av", bufs=2, space="PSUM") as ps_av:

            # resident tensors
            qT_sb = [pres.tile([P, S], BF16, name=f"qT{p}")
                     for p in range(NPAIR)]
            kT_sb = [pres.tile([P, S], BF16, name=f"kT{p}")
                     for p in range(NPAIR)]
            v_sb = pres.tile([P, NKT, 8, 65], BF16)
            attn_sb = pres.tile([P, NPAIR, S], BF16)
            # resident x: one DMA per block for the whole kernel (the
            # baseline re-streamed x per pair — 4x the DMA — which starved
            # pair 0's exp stream); xv quarters are also shared by the two
            # half-width v projection passes
            xk_res = [pres.tile([P, KT, DG], BF16, name=f"xkr{nb}")
                      for nb in range(4)]
            xq_res = [pres.tile([P, KT, DG], BF16, name=f"xqr{nb}")
                      for nb in range(4)]
            xv_res = [pres.tile([P, KT, DG], BF16, name=f"xvr{nb}")
                      for nb in range(4)]

            # weights: wq/wk/wv are dead after pair 1 and wo is only
            # needed from pair 3, so 3 rotating slots cover all four
            w_dram = {"q": wqT, "k": wkT, "v": wvT}
            w_sb = {}

            def ensure_w(key, half=None):
                """Allocate w tile on first call; DMA kt-rows [half].

                kt-halved loads match the projection chains' kt 0..3 /
                4..7 split, so a chain's first half starts after 0.5MB —
                the region tracker scopes each matmul's wait to the kt
                rows it reads.
                """
                if key == "o":
                    if key not in w_sb:
                        t = pw.tile([P, NPAIR, DM], BF16, tag="w", name="wo")
                        nc.sync.dma_start(t[:], woT[:, :])
                        w_sb[key] = t
                    return
                if key not in w_sb:
                    w_sb[key] = pw.tile([P, KT, DG], BF16, tag="w",
                                        name=f"w{key}")
                lo, hi = (0, KT) if half is None else \
                    (half * (KT // 2), (half + 1) * (KT // 2))
                nc.sync.dma_start(
                    w_sb[key][:, lo:hi, :],
                    w_dram[key][:, lo * DG:hi * DG])

            xres = {"q": xq_res, "k": xk_res, "v": xv_res}
            xdram = {"q": xqT, "k": xkT, "v": xvT}

            def load_xres(proj, nb, half=None):
                lo, hi = (0, KT) if half is None else \
                    (half * (KT // 2), (half + 1) * (KT // 2))
                nc.sync.dma_start(
                    xres[proj][nb][:, lo:hi, :],
                    xdram[proj][:, nb * KT * DG + lo * DG:
                                nb * KT * DG + hi * DG],
                )

            nc.vector.memset(v_sb[:, :, :, 64:65], 1.0)

            # warm the ACT exp table-set during the DMA-bound head so the
            # ~1.3us ACT_TABLE_LOAD is off the first real exp's latency
            warm = pres.tile([1, 1], F32, name="actwarm")
            nc.vector.memset(warm[:], 0.0)
            nc.scalar.activation(warm[0:1, 0:1], warm[0:1, 0:1], AF.Exp)

            if has_bias:
                x9 = pres.tile([P, DG], BF16)      # ones row, rest zero
                xv9 = pres.tile([P, P], BF16)
                w9 = {
                    "q": pres.tile([P, DG], BF16, name="w9q"),
                    "k": pres.tile([P, DG], BF16, name="w9k"),
                    "v": pres.tile([P, DG], BF16, name="w9v"),
                }
                for t in (x9, xv9, w9["q"], w9["k"], w9["v"]):
                    nc.vector.memset(t[:], 0.0)
                nc.vector.memset(x9[0:1, :], 1.0)
                nc.vector.memset(xv9[0:1, :], 1.0)
                for key, d in (("q", bq), ("k", bk), ("v", bv)):
                    nc.sync.dma_start(w9[key][0:1, :], d[:])

            emitted = set()
            queued = set()
            pending = []          # deferred emitters, dripped between groups

            def qk_first(proj, nb, p):
                """First half (kt 0..3) of a q/k projection chain."""
                xt = xres[proj][nb]
                ps = ps_proj.tile([P, DG], F32, tag="pp",
                                  name=f"ps_{proj}{nb}_{p}")
                for kt in range(KT // 2):
                    nc.tensor.matmul(
                        ps[:], w_sb[proj][:, kt, p * P:(p + 1) * P],
                        xt[:, kt, :],
                        start=(kt == 0), stop=False,
                    )
                return ps

            def qk_second(proj, nb, p, ps):
                """Second half (kt 4..7) + evacuation."""
                dst = qT_sb if proj == "q" else kT_sb
                xt = xres[proj][nb]
                for kt in range(KT // 2, KT):
                    nc.tensor.matmul(
                        ps[:], w_sb[proj][:, kt, p * P:(p + 1) * P],
                        xt[:, kt, :],
                        start=False,
                        stop=(kt == KT - 1 and not has_bias),
                    )
                if has_bias:
                    nc.tensor.matmul(
                        ps[:], w9[proj][:, p * P:(p + 1) * P], x9[:],
                        start=False, stop=True,
                    )
                nc.vector.tensor_copy(dst[p][:, nb * DG:(nb + 1) * DG], ps[:])

            def qk_subblock(proj, nb, p):
                """Project q or k for seq block nb, one pair."""
                ps = qk_first(proj, nb, p)
                qk_second(proj, nb, p, ps)

            def v_block(half, m):
                """Project v for key tile m, 4 heads (half 0: 0-3, 1: 4-7)."""
                xt = xv_res[m // 4]
                sl = slice((m % 4) * P, (m % 4 + 1) * P)
                lo = half * 4 * 64
                ps = ps_proj.tile([P, 4 * 64], F32, tag="pp",
                                  name=f"ps_v{half}_{m}")
                for kt in range(KT):
                    nc.tensor.matmul(
                        ps[:], xt[:, kt, sl],
                        w_sb["v"][:, kt, lo:lo + 4 * 64],
                        start=(kt == 0),
                        stop=(kt == KT - 1 and not has_bias),
                    )
                if has_bias:
                    nc.tensor.matmul(ps[:], xv9[:],
                                     w9["v"][:, lo:lo + 4 * 64],
                                     start=False, stop=True)
                nc.vector.tensor_copy(
                    v_sb[:, m, 4 * half:4 * half + 4, 0:64],
                    ps[:].rearrange("p (h d) -> p h d", d=64),
                )

            def oproj_pre(m, n):
                """Accumulate pairs 0..2 (normalized a full pair-cycle ago)."""
                ensure_w("o")
                ps = ps_proj.tile([P, DG], F32, tag="pp", name=f"ps_o{m}_{n}")
                for kp in range(NPAIR - 1):
                    nc.tensor.matmul(
                        ps[:], attn_sb[:, kp, m * P:(m + 1) * P],
                        w_sb["o"][:, kp, n * DG:(n + 1) * DG],
                        start=(kp == 0), stop=False,
                    )
                return ps

            def oproj_fin(m, n, ps):
                """Pair 3 (waits on the fresh normalization) + evacuation."""
                kp = NPAIR - 1
                nc.tensor.matmul(
                    ps[:], attn_sb[:, kp, m * P:(m + 1) * P],
                    w_sb["o"][:, kp, n * DG:(n + 1) * DG],
                    start=False, stop=True,
                )
                ot = pout.tile([P, DG], BF16, tag="ot", name=f"ot{m}_{n}")
                nc.vector.tensor_copy(ot[:], ps[:])
                nc.sync.dma_start(
                    out[m * P:(m + 1) * P, n * DG:(n + 1) * DG], ot[:])

            pending_o = []        # oproj items, dripped after AV

            def queue(key):
                if key not in emitted and key not in queued:
                    queued.add(key)
                    pending.append(key)

            def ensure(key):
                if key in emitted:
                    return
                # a half-open chain shares ps_proj bufs; emitting anything
                # else from the pool in between would recycle its bank
                flush_half()
                emitted.add(key)
                kind = key[0]
                if kind == "v":
                    v_block(key[1], key[2])
                else:
                    qk_subblock(*key)

            half_open = []        # qk chain with only kt 0..3 emitted

            def flush_half():
                while half_open:
                    key, ps = half_open.pop(0)
                    qk_second(*key, ps)

            def drip_qk(full=False):
                # chains drip in 4-matmul halves (~850ns) so a drip never
                # overruns the exp window and delays the AV matmuls behind
                # it; full chains where other ps_proj users interleave
                if half_open:
                    key, ps = half_open.pop(0)
                    qk_second(*key, ps)
                    return
                while pending:
                    key = pending.pop(0)
                    if key in emitted:
                        continue
                    emitted.add(key)
                    if full:
                        qk_subblock(*key)
                    else:
                        ps = qk_first(*key)
                        half_open.append((key, ps))
                    return

            o_open = []           # oproj chains with pairs 0..2 emitted

            def drip_o(allow_fin=True):
                if o_open and allow_fin and (len(o_open) >= 2
                                             or not pending_o):
                    m, n, ps = o_open.pop(0)
                    oproj_fin(m, n, ps)
                    return
                if pending_o and len(o_open) < 2:
                    m, n = pending_o.pop(0)
                    o_open.append((m, n, oproj_pre(m, n)))

            def oproj_chunk(qb):
                """Queue output projection for seq rows qb*512..+512."""
                for mi in range(4):
                    for n in range(2):
                        pending_o.append((4 * qb + mi, n))

            def scores_mm(p, qb, g):
                sc = ps_sc.tile([P, 2, DG], F32, tag="sc",
                                name=f"sc{p}_{qb}_{g}")
                for h in range(2):
                    nc.tensor.matmul(
                        sc[:, h, :],
                        kT_sb[p][64 * h:64 * h + 64, g * P:(g + 1) * P],
                        qT_sb[p][64 * h:64 * h + 64, qb * DG:(qb + 1) * DG],
                        start=True, stop=True,
                        tile_position=(64 * h, 0),
                    )
                return sc

            # priority DMA order (descriptors stream in-order, so byte
            # position IS arrival time): the k and q projection chains
            # split at kt=4, so interleaving w/x kt-halves lets the first
            # chain halves start after 1MB instead of 4MB; then wv + the
            # xk/xv blocks in just-in-time consumption order; xq1+ rides
            # behind (xq(qb) is only needed +18us per qb)
            ensure_w("k", 0)
            load_xres("k", 0, 0)
            ensure_w("k", 1)
            load_xres("k", 0, 1)
            ensure_w("q", 0)
            load_xres("q", 0, 0)
            ensure_w("q", 1)
            load_xres("q", 0, 1)
            ensure_w("v", 0)
            ensure_w("v", 1)
            load_xres("v", 0)
            load_xres("k", 1)
            load_xres("v", 1)
            load_xres("k", 2)
            load_xres("v", 2)
            load_xres("k", 3)
            load_xres("v", 3)
            load_xres("q", 1)

            hoisted = {}

            # attention: pair-outer, q-block, one key tile per group.
            # scores(g+1) are emitted before AV(g) so the PE computes them
            # under exp(g) and the exp cadence stays at the ACT floor.
            for p in range(NPAIR):
                if p == NPAIR - 1:
                    ensure_w("o")     # wo DMA ahead of the first oproj drip
                vhalf = p if p < 2 else None   # p0: heads 0-3, p1: heads 4-7
                for qb in range(NQB):
                    flush_half()
                    if p == 0 and qb in (1, 2):
                        # deferred resident xq loads: emitted only now so
                        # they never compete with the JIT-critical xv/xk
                        # streams in the head window
                        load_xres("q", qb + 1)
                    if p == 0 and qb == 0:
                        # k chain first: its DMAs land first, and the PE
                        # queue is in-order — q-first would stall the PE
                        # on xq while k's data sits ready
                        ensure(("k", 0, 0))
                    ensure(("q", qb, p))
                    if qb + 1 < NQB:
                        # pair 0's q chains are not pre-dripped: their xq
                        # blocks arrive late, and a dripped chain waiting on
                        # DMA at the head of the in-order PE queue would
                        # block everything behind it
                        if p > 0:
                            queue(("q", qb + 1, p))
                    elif p + 1 < NPAIR:
                        queue(("q", 0, p + 1))
                    if qb == NQB - 1 and p + 1 < NPAIR:
                        for nb in range(4):
                            queue(("k", nb, p + 1))
                    av = [
                        ps_av.tile([65, DG], F32, tag="av",
                                   name=f"av{p}_{qb}_{h}")
                        for h in range(2)
                    ]
                    ensure(("k", 0, p))
                    if vhalf is not None:
                        ensure(("v", vhalf, 0))
                    sc_cur = hoisted.pop((p, qb), None)
                    if sc_cur is None:
                        sc_cur = scores_mm(p, qb, 0)
                    for g in range(NKT):
                        if g + 1 < NKT:
                            ensure(("k", (g + 1) // 4, p))
                            if vhalf is not None:
                                ensure(("v", vhalf, g + 1))
                        ex = pex.tile([P, 2, DG], BF16, tag="ex",
                                      name=f"ex{p}_{qb}_{g}")
                        nc.scalar.activation(ex[:], sc_cur[:], AF.Exp,
                                             scale=0.125)
                        rhs = [ex[:, 0, :], ex[:, 1, :]]
                        if g + 1 < NKT:
                            sc_cur = scores_mm(p, qb, g + 1)
                        else:
                            # hoist the NEXT block's first scores group in
                            # front of this block's boundary work, so the
                            # exp stream crosses the boundary without a
                            # bubble (possible whenever the next block's
                            # q/k chains have already dripped through)
                            nxt = ((p, qb + 1) if qb + 1 < NQB
                                   else (p + 1, 0) if p + 1 < NPAIR
                                   else None)
                            if (nxt is not None
                                    and ("q", nxt[1], nxt[0]) in emitted
                                    and ("k", 0, nxt[0]) in emitted):
                                hoisted[nxt] = scores_mm(nxt[0], nxt[1], 0)
                        # one qk drip BEFORE the AV matmuls: AV(g) waits on
                        # exp(g) (ACT, ~1.1us) at the head of the in-order
                        # PE queue, so this fills the stall with projection
                        # work; oproj drips stay after AV (their LDW waits
                        # on fresh normalization). p<=1 drips full chains:
                        # v_blocks interleave there and would recycle a
                        # half-open chain's psum bank
                        drip_qk(full=(p <= 1))
                        for h in range(2):
                            nc.tensor.matmul(
                                av[h][:],
                                v_sb[:, g, 2 * p + h, :],
                                rhs[h],
                                start=(g == 0),
                                stop=(g == NKT - 1),
                            )
                        # pre-chains from g >= 3; pair-3 finishes from
                        # g >= 5, by which the previous block's
                        # normalization chain has landed — so no oproj
                        # matmul ever heads the PE queue waiting on it
                        if p == NPAIR - 1 and g >= 3:
                            drip_o(allow_fin=(g >= 5))
                            drip_o(allow_fin=(g >= 5))
                    # boundary: evacuate unnormalized attn (freeing the av
                    # banks early — the next block's first AV otherwise
                    # stalls the whole in-order PE queue on this chain),
                    # then normalize: 1-lane approx reciprocal on the sums
                    # row (the matmul ones-column), gpsimd partition-
                    # broadcast, multiply in place.  The LAST block fuses
                    # the evacuation into the normalize multiply instead
                    # (no next block — av lifetime is free) to shorten the
                    # exposed tail chain.
                    last = (p == NPAIR - 1 and qb == NQB - 1)
                    for h in range(2):
                        if not last:
                            nc.vector.tensor_copy(
                                attn_sb[64 * h:64 * h + 64, p,
                                        qb * DG:(qb + 1) * DG],
                                av[h][0:64, :],
                            )
                        s0 = psmall.tile([1, DG], F32, tag="s0",
                                         name=f"s0_{qb}_{p}_{h}")
                        nc.vector.tensor_scalar_mul(
                            s0[0:1, :], av[h][64:65, :], 1.0)
                        r1 = psmall.tile([1, DG], F32, tag="r1",
                                         name=f"r1_{qb}_{p}_{h}")
                        with nc.allow_low_precision(
                                reason="softmax denominators, ~51 ULP"):
                            nc.vector.reciprocal_approx_fast(
                                r1[0:1, :], s0[0:1, :])
                        rbc = psmall.tile([P, DG], F32, tag="rbc",
                                          name=f"rbc{qb}_{p}_{h}")
                        nc.gpsimd.partition_broadcast(rbc[:], r1[0:1, :])
                        sl = attn_sb[64 * h:64 * h + 64, p,
                                     qb * DG:(qb + 1) * DG]
                        if last:
                            nc.vector.tensor_tensor(
                                sl, av[h][0:64, :], rbc[0:64, :],
                                mybir.AluOpType.mult)
                        else:
                            nc.vector.tensor_tensor(
                                sl, sl, rbc[64 * h:64 * h + 64, :],
                                mybir.AluOpType.mult)
                    if p == NPAIR - 1:
                        oproj_chunk(qb)
            while pending or half_open:
                drip_qk()
            # tail: pre-chains (pairs 0..2) have no dependency on the final
            # normalization, so open as many as PSUM allows — the two
            # ps_proj slots plus two chains borrowed from the now-dead
            # scores banks — and emit them all ahead of the fins.  This
            # keeps the PE warm through the last normalization chain
            # (HAM re-throttles after ~3.4us idle) and lets the fins
            # stream back-to-back once it lands.
            def tail_pre(m, n):
                sc_t = ps_sc.tile([P, 2, DG], F32, tag="sc",
                                  name=f"osc{m}_{n}")
                ps = sc_t[:, 0, :]
                ensure_w("o")
                for kp in range(NPAIR - 1):
                    nc.tensor.matmul(
                        ps, attn_sb[:, kp, m * P:(m + 1) * P],
                        w_sb["o"][:, kp, n * DG:(n + 1) * DG],
                        start=(kp == 0), stop=False,
                    )
                return ps

            while pending_o and len(o_open) < 2:
                m, n = pending_o.pop(0)
                o_open.append((m, n, oproj_pre(m, n)))
            n_sc = 0
            while pending_o and n_sc < 2:
                m, n = pending_o.pop(0)
                o_open.append((m, n, tail_pre(m, n)))
                n_sc += 1
            while o_open:
                m, n, ps = o_open.pop(0)
                oproj_fin(m, n, ps)
                if pending_o:
                    m2, n2 = pending_o.pop(0)
                    o_open.append((m2, n2, tail_pre(m2, n2)))
    nc.compile()
    return nc


_CACHE = {}


def _get_nc(has_bias):
    if has_bias not in _CACHE:
        _CACHE[has_bias] = _build(has_bias)
    return _CACHE[has_bias]


def _pack_x(a):
    """[S, DM] -> partition-major [P, nb, kt, 512] (flattened)."""
    t = np.asarray(a, np.float32).T.astype(ml_dtypes.bfloat16)  # [DM, S]
    return np.ascontiguousarray(
        t.reshape(KT, P, 4, DG).transpose(1, 2, 0, 3).reshape(P, 4 * KT * DG))


def _pack_w(w):
    """W slice [DG, DM] -> partition-major [P, kt, 512] (flattened)."""
    t = np.asarray(w, np.float32).T.astype(ml_dtypes.bfloat16)  # [DM, DG]
    return np.ascontiguousarray(
        t.reshape(KT, P, DG).transpose(1, 0, 2).reshape(P, KT * DG))


def _pack_wo(w):
    """Wo column slice [DM, DG] -> partition-major [P, kp, DM]."""
    t = np.asarray(w, np.float32).T.astype(ml_dtypes.bfloat16)  # [DG, DM]
    return np.ascontiguousarray(
        t.reshape(NPAIR, P, DM).transpose(1, 0, 2).reshape(P, NPAIR * DM))


def _run(Q, K, V, Wq, bq, Wk, bk, Wv, bv, Wo, bo, trace=False):
    Q, K, V = (np.asarray(t, np.float32) for t in (Q, K, V))
    Wq, Wk, Wv, Wo = (np.asarray(t, np.float32) for t in (Wq, Wk, Wv, Wo))
    bq, bk, bv, bo = (np.asarray(t, np.float32) for t in (bq, bk, bv, bo))
    B = Q.shape[0]
    has_bias = bool(np.any(bq) or np.any(bk) or np.any(bv))
    nc = _get_nc(has_bias)

    xts = [(_pack_x(Q[b]), _pack_x(K[b]), _pack_x(V[b])) for b in range(B)]
    wts = []
    for g in range(2):
        sl = slice(DG * g, DG * (g + 1))
        wts.append({
            "wqT": _pack_w(Wq[sl]), "wkT": _pack_w(Wk[sl]),
            "wvT": _pack_w(Wv[sl]),
            "woT": _pack_wo(Wo[:, sl]),
            "bq": np.ascontiguousarray(bq[None, sl]).astype(ml_dtypes.bfloat16),
            "bk": np.ascontiguousarray(bk[None, sl]).astype(ml_dtypes.bfloat16),
            "bv": np.ascontiguousarray(bv[None, sl]).astype(ml_dtypes.bfloat16),
        })
    in_maps = []
    for c in range(8):
        b, g = c // 2, c % 2
        m = {
            "xqT": xts[b][0], "xkT": xts[b][1], "xvT": xts[b][2],
            "wqT": wts[g]["wqT"], "wkT": wts[g]["wkT"],
            "wvT": wts[g]["wvT"], "woT": wts[g]["woT"],
        }
        if has_bias:
            m["bq"] = wts[g]["bq"]
            m["bk"] = wts[g]["bk"]
            m["bv"] = wts[g]["bv"]
        in_maps.append(m)

    res = run_bass_kernel_spmd(nc, in_maps, core_ids=list(range(8)),
                               trace=trace)
    outp = np.empty((B, S, DM), np.float32)
    for b in range(B):
        outp[b] = (res.results[2 * b]["out"].astype(np.float32)
                   + res.results[2 * b + 1]["out"].astype(np.float32))
    outp += bo[None, None, :]
    return outp, res


def kernel(Q, K, V, Wq, bq, Wk, bk, Wv, bv, Wo, bo):
    outp, _ = _run(Q, K, V, Wq, bq, Wk, bk, Wv, bv, Wo, bo, trace=False)
    return outp
